# revision 54
# baseline (speedup 1.0000x reference)
"""Multi-head causal attention (B=2, S=2048, D=1024, H=16) on 8 TRN2 NeuronCores.

Sharding: core c handles batch b = c//4 and head-group g = c%4 (4 heads, 256 dims).
Each core computes Q/K/V projections for its head group from x[b], runs causal
attention per head, and applies its 256 rows of Wo, producing a partial [S, D]
output (bf16). The host sums the 4 head-group partials per batch in fp32.

Device algorithm (per core); matmul operands bf16, accumulation fp32 in PSUM:
  warm-up matmul burst at body start un-throttles the PE HAM clock gate
  (kept small: long garbage bursts push the chip into the P0 power state
  and slow EVERY engine ~19%)
  qT/kT = Wq_g^T @ x^T, stored [64*2, pair, S] (head dims on partitions)
  v     = x @ Wv_g, stored per 128-seq block with an appended ones column
  attention runs per head-pair, both pairs ascending chunk order, with the two
  heads interleaved per 512-wide i-chunk:
    S^T[j,i] strips via matmul(lhsT=kT_block, rhs=qT_chunk); the two heads'
    matmuls are issued back-to-back on disjoint PE row groups (K=64 row
    pairing) so they run concurrently; diagonal strips narrowed to the
    causally-valid column range
    P~^T = exp(scale * S^T) (ScalarE, 2 strips per instruction), diagonal
    blocks masked with an upper-triangular 0/1 multiply
    O'^T[65, i] += V'_j^T @ P~^T_j  (PSUM accumulate; row 64 = softmax denom)
    per chunk, both heads: denominators copied out with the numerators,
    reciprocal'd lane-parallel via a DRAM reshape bounce, broadcast with a
    stride-0 DRAM read, then O^T = num * recip (DVE); head 1 of the pair
    lands at partitions 0-63 and is shifted to 64-127 with a SBUF->SBUF DMA
  y = O @ Wo_g (lhsT = O^T tiles), PSUM copied out bf16, DMA out.

All TensorE work besides the attention strips is interleaved as "fillers"
between strip groups: pair-0 windows consume the pair-0 Q/K/V projections
(with forced drains before the chunk that needs them); pair-1 windows consume
the pair-1 Q/K projections (drained on demand per chunk) and the output
projection, emitted per chunk as soon as that chunk's oT is normalized. For
the final chunk the pair-0 half of the output projection is pre-staged under
its strip windows, so the post-attention tail is only that chunk's normalize
chain plus the pair-1 matmuls, adds, and split-queue y DMAs.

Queue discipline (the big scheduling hazards found by tracing):
  - every DMA issue costs ~0.7us of the issuing engine's sequencer and ring
    backpressure can block that sequencer outright, so the scalar queue
    (whose sequencer carries the exp stream) gets no bulk loads: x rides
    sync, weights ride GpSimd SWDGE; scalar only takes tail-phase DMAs
    after the exp stream has drained
  - ScalarE/DVE are strict FIFOs: a copy that waits on late PE work blocks
    everything behind it, so mid-stream PSUM->SBUF copies stay on DVE and
    ScalarE only takes copies in exp-idle slots (the tail)
"""

import os
from collections import deque

import ml_dtypes
import numpy as np

import concourse.bass as bass
import concourse.mybir as mybir
import concourse.tile as tile
from concourse.bass_utils import run_bass_kernel_spmd
from concourse.masks import make_upper_triangular

F32 = mybir.dt.float32
BF16 = mybir.dt.bfloat16

B, S, D, H = 2, 2048, 1024, 16
HD = 64                     # head dim
GH = 4                      # heads per core
GC = GH * HD                # 256 projection cols per core
P = 128
KD = D // P                 # 8 contraction chunks for projections
NSB = S // P                # 16 seq blocks
CHW = 512                   # i-chunk width
NCH = S // CHW              # 4 i-chunks
SCALE = HD ** -0.5
NWARM = 40                  # HAM warm-up matmuls (~4.3us cold = one SHORT window)

_NC_CACHE = None
LAST_RESULTS = None         # BassKernelResults of the most recent run (for test.py)


class _Fillers:
    """Queue of small emission closures (1-2 TensorE ops each) drained
    between attention strip groups to keep the PE busy while ScalarE
    works through the exp stream. Markers let the consumer force-drain
    the prefix a dependent phase needs."""

    def __init__(self):
        self.q = deque()
        self.seen = set()

    def add(self, fn):
        self.q.append(fn)

    def add_marker(self, key):
        self.q.append(key)

    def _emit_one(self):
        item = self.q.popleft()
        if callable(item):
            item()
            return None
        self.seen.add(item)
        return item

    def step(self, n):
        done = 0
        while done < n and self.q:
            if self._emit_one() is None:
                done += 1

    def drain_until(self, key):
        if key in self.seen:
            return
        while self.q:
            if self._emit_one() == key:
                return

    def drain(self):
        while self.q:
            self._emit_one()


class _FillerChain:
    """step() prefers the primary queue, falls back to the secondary."""

    def __init__(self, primary, secondary):
        self.primary = primary
        self.secondary = secondary

    def step(self, n):
        done = 0
        while done < n:
            if self.primary.q:
                if self.primary._emit_one() is None:
                    done += 1
            elif self.secondary.q:
                if self.secondary._emit_one() is None:
                    done += 1
            else:
                return


def _emit_pair_attention(tc, pair, pools, tensors, fillers, emit_outproj,
                         pre_chunk=None, pre_group=None):
    nc = tc.nc
    ps_sc, ps_pv, ps_fill, dpool, ppool, npool, opool = pools
    qT, kT, v_sb, oT, trimask, ones_row = tensors

    for c in range(NCH):
        njb = 4 * c + 4
        if pre_chunk is not None:
            pre_chunk(c)
        pvacc0 = ps_pv.tile([HD + 1, CHW], F32, tag="pv0", name="pvacc0")
        pvacc1 = ps_pv.tile([HD + 1, CHW], F32, tag="pv1", name="pvacc1")
        pvacc = {0: pvacc0, 1: pvacc1}
        # strip tasks, heads interleaved so paired score matmuls are adjacent
        tasks = [(hp, jb) for jb in range(njb) for hp in (0, 1)]
        for g0 in range(0, len(tasks), 2):
            group = tasks[g0:g0 + 2]
            if pre_group is not None:
                for _, jb in group:
                    pre_group(jb)
            sc = ps_sc.tile([P, 2, CHW], F32, tag="sc")
            pt = ppool.tile([P, 2, CHW], BF16, tag="pt")
            for t, (hp, jb) in enumerate(group):
                bp = hp * HD
                tl = max(0, jb - 4 * c) * P
                nc.tensor.matmul(
                    sc[:, t, tl:],
                    kT[bp:bp + HD, pair, jb * P:(jb + 1) * P],
                    qT[bp:bp + HD, pair, c * CHW + tl:(c + 1) * CHW])
            tlg = max(0, group[0][1] - 4 * c) * P
            nc.scalar.activation(
                pt[:, :len(group), tlg:], sc[:, :len(group), tlg:],
                mybir.ActivationFunctionType.Exp, scale=SCALE)
            for t, (hp, jb) in enumerate(group):
                if jb >= 4 * c:               # diagonal block: causal mask
                    tl = (jb - 4 * c) * P
                    nc.vector.tensor_mul(
                        pt[:, t, tl:tl + P], pt[:, t, tl:tl + P], trimask)
            for t, (hp, jb) in enumerate(group):
                h = pair * 2 + hp
                tl = max(0, jb - 4 * c) * P
                nc.tensor.matmul(
                    pvacc[hp][:, tl:], v_sb[:, jb, h, :], pt[:, t, tl:],
                    start=(jb == 0), stop=(jb == njb - 1))
            fillers.step(7)

        if pair == 1 and c == NCH - 1:
            # the normalize chain below is the only thing between the last
            # strip and the final outproj burst; keep the PE's HAM clock
            # gate open across that stall with a few dependency-free matmuls
            wt = ps_fill.tile([P, CHW], F32, tag="fill", name="tailwarm")
            for w in range(24):
                nc.tensor.matmul(
                    wt[:, (w % 4) * P:(w % 4 + 1) * P], trimask, trimask,
                    start=True, stop=True)

        # per-chunk normalize for both heads: copy num/denom out of PSUM,
        # lane-parallel reciprocal via DRAM reshape, broadcast, multiply
        dden = dpool.tile([2, CHW], F32, tag="dden")
        onums = {}
        for hp in (0, 1):
            onum = opool.tile([HD + 1, CHW], F32, tag=f"on{hp}")
            nc.vector.tensor_copy(out=onum, in_=pvacc[hp])
            nc.sync.dma_start(
                out=dden[hp:hp + 1, :], in_=onum[HD:HD + 1, :])
            onums[hp] = onum
        nel = 2 * CHW // P                    # 8 elems/lane
        rv = npool.tile([P, nel], F32, tag="recp")
        nc.sync.dma_start(out=rv, in_=bass.AP(
            tensor=dden.tensor, offset=dden.offset, ap=[[nel, P], [1, nel]]))
        nc.vector.reciprocal(out=rv, in_=rv)
        drec = dpool.tile([2, CHW], F32, tag="drec")
        nc.sync.dma_start(out=bass.AP(
            tensor=drec.tensor, offset=drec.offset,
            ap=[[nel, P], [1, nel]]), in_=rv)
        cs = slice(c * CHW, (c + 1) * CHW)
        # after the last exp the scalar queue is free: let the final chunk's
        # broadcast/shift DMAs ride it in parallel with the sync queue
        at_tail = (pair == 1 and c == NCH - 1)
        for hp in (0, 1):
            bcr = npool.tile([HD, CHW], F32, tag="bcr")
            eng = nc.scalar if (at_tail and hp == 1) else nc.sync
            eng.dma_start(out=bcr, in_=bass.AP(
                tensor=drec.tensor, offset=drec.offset + hp * CHW,
                ap=[[0, HD], [1, CHW]]))
            if hp == 0:
                nc.vector.tensor_mul(
                    oT[0:HD, pair, cs], onums[hp][0:HD, :], bcr)
            else:
                tmp = npool.tile([HD, CHW], BF16, tag="otmp")
                nc.vector.tensor_mul(tmp, onums[hp][0:HD, :], bcr)
                eng2 = nc.scalar if at_tail else nc.sync
                eng2.dma_start(out=oT[HD:P, pair, cs], in_=tmp)
        if emit_outproj is not None:
            emit_outproj(c)


def _emit(tc):
    nc = tc.nc
    xT = nc.dram_tensor("xT", [D, S], BF16, kind="ExternalInput")
    wq = nc.dram_tensor("wq", [D, GC], BF16, kind="ExternalInput")
    wk = nc.dram_tensor("wk", [D, GC], BF16, kind="ExternalInput")
    wv = nc.dram_tensor("wv", [D, GC], BF16, kind="ExternalInput")
    wo = nc.dram_tensor("wo", [GC, D], BF16, kind="ExternalInput")
    y = nc.dram_tensor("y", [S, D], BF16, kind="ExternalOutput")

    xT_t = xT[:].rearrange("(o p) s -> p o s", p=P)      # [128, 8, S]
    wq_t = wq[:].rearrange("(o p) c -> p o c", p=P)      # [128, 8, 256]
    wk_t = wk[:].rearrange("(o p) c -> p o c", p=P)
    wv_t = wv[:].rearrange("(o p) c -> p o c", p=P)
    wo_t = wo[:].rearrange("(o p) n -> p o n", p=P)      # [128, 2, 1024]

    from contextlib import ExitStack

    with ExitStack() as top:
        persist = top.enter_context(tc.tile_pool(name="persist", bufs=1))

        trimask = persist.tile([P, P], BF16)             # 1.0 where j<=i else 0
        make_upper_triangular(nc, trimask, val=1.0, diag=True)
        ones_bf = persist.tile([P, 1], BF16)
        nc.vector.memset(ones_bf, 1.0)
        ones_row = persist.tile([1, HD], BF16)           # outer-product lhsT
        nc.vector.memset(ones_row, 1.0)

        wq_sb = persist.tile([P, KD, GC], BF16)
        wk_sb = persist.tile([P, KD, GC], BF16)
        wv_sb = persist.tile([P, KD, GC], BF16)
        wo_sb = persist.tile([P, 2, D], BF16)
        xfull = persist.tile([P, KD, S], BF16)
        # x rides the sync queue, weights ride GpSimd SWDGE, and the scalar
        # queue carries NO bulk loads: DMA-ring backpressure blocks the
        # issuing sequencer, and the scalar sequencer must reach the exp
        # stream quickly
        for g in range(KD // 2):
            nc.sync.dma_start(
                out=xfull[:, 2 * g:2 * g + 2, 0:CHW],
                in_=xT_t[:, 2 * g:2 * g + 2, 0:CHW])
        for half in range(2):
            ks = slice(4 * half, 4 * half + 4)
            nc.gpsimd.dma_start(out=wq_sb[:, ks, :], in_=wq_t[:, ks, :])
            nc.gpsimd.dma_start(out=wk_sb[:, ks, :], in_=wk_t[:, ks, :])
        for half in range(2):
            ks = slice(4 * half, 4 * half + 4)
            nc.gpsimd.dma_start(out=wv_sb[:, ks, :], in_=wv_t[:, ks, :])
        for ch in range(1, NCH):
            for g in range(KD // 2):
                nc.sync.dma_start(
                    out=xfull[:, 2 * g:2 * g + 2, ch * CHW:(ch + 1) * CHW],
                    in_=xT_t[:, 2 * g:2 * g + 2, ch * CHW:(ch + 1) * CHW])
        nc.gpsimd.dma_start(out=wo_sb[:, 0:1, :], in_=wo_t[:, 0:1, :])
        nc.gpsimd.dma_start(out=wo_sb[:, 1:2, :], in_=wo_t[:, 1:2, :])

        qT = persist.tile([P, 2, S], BF16)               # [pair-cols, pair, seq]
        kT = persist.tile([P, 2, S], BF16)
        v_sb = persist.tile([P, NSB, GH, HD + 1], BF16)  # ones col appended
        oT = persist.tile([P, 2, S], BF16)
        nc.vector.tensor_copy(
            out=v_sb[:, :, :, HD:HD + 1],
            in_=ones_bf[:, 0:1].to_broadcast((P, NSB, GH, 1)))

        tensors = (qT, kT, v_sb, oT, trimask, ones_row)

        # ---- attention with all projections as ordered fillers ----
        with ExitStack() as ph_b:
            ps_sc = ph_b.enter_context(
                tc.tile_pool(name="ps_sc", bufs=2, space="PSUM"))
            ps_pv = ph_b.enter_context(
                tc.tile_pool(name="ps_pv", bufs=1, space="PSUM"))
            ps_fill = ph_b.enter_context(
                tc.tile_pool(name="ps_fill", bufs=2, space="PSUM"))
            dpool = ph_b.enter_context(
                tc.tile_pool(name="dscr", bufs=4, space="DRAM"))
            ppool = ph_b.enter_context(tc.tile_pool(name="pstrip", bufs=5))
            npool = ph_b.enter_context(tc.tile_pool(name="norm", bufs=6))
            opool = ph_b.enter_context(tc.tile_pool(name="onum", bufs=3))
            ypool = ph_b.enter_context(tc.tile_pool(name="ystage", bufs=2))
            ypool3 = ph_b.enter_context(tc.tile_pool(name="ystage3", bufs=4))
            pools = (ps_sc, ps_pv, ps_fill, dpool, ppool, npool, opool)

            # HAM warm-up: ~4us of back-to-back cold matmuls so the PE clock
            # gate opens before the real projection stream begins
            wtile = ps_fill.tile([P, CHW], F32, tag="fill", name="warm")
            for w in range(NWARM):
                nc.tensor.matmul(
                    wtile[:, (w % 4) * P:(w % 4 + 1) * P], trimask, trimask,
                    start=True, stop=True)
            # preload the Exp activation table during the DMA lead-in so the
            # first real exp doesn't pay the ~1.3us table load
            etab = npool.tile([P, P], BF16, tag="etab")
            nc.scalar.activation(
                etab, trimask, mybir.ActivationFunctionType.Exp, scale=1.0)

            f0 = _Fillers()

            def _proj_chunk(which, pair_, ch):
                # which: 0=Q, 1=K; emits 8 accumulating matmuls + copy-out.
                # pair-1 copies land at chunk-boundary drains where the exp
                # FIFO has natural slack, so they ride ScalarE to relieve
                # DVE, which co-paces pair-1.
                cell = {}
                w_sb = wq_sb if which == 0 else wk_sb
                dst = qT if which == 0 else kT

                def alloc_mm(k, cell=cell, ch=ch, w_sb=w_sb, pair_=pair_):
                    if k == 0:
                        cell["p"] = ps_fill.tile(
                            [P, CHW], F32, tag="fill", name="fillqk")
                    nc.tensor.matmul(
                        cell["p"], w_sb[:, k, pair_ * P:(pair_ + 1) * P],
                        xfull[:, k, ch * CHW:(ch + 1) * CHW],
                        start=(k == 0), stop=(k == KD - 1))

                def copy(cell=cell, ch=ch, dst=dst, pair_=pair_):
                    nc.vector.tensor_copy(
                        out=dst[:, pair_, ch * CHW:(ch + 1) * CHW],
                        in_=cell["p"])

                for k in range(KD):
                    f0.add(lambda k=k: alloc_mm(k))
                f0.add(copy)

            def _v_block(sb):
                cell = {}

                def alloc_mm(k, cell=cell, sb=sb):
                    if k == 0:
                        cell["pv"] = ps_fill.tile(
                            [P, CHW], F32, tag="fill", name="fillpv")
                    nc.tensor.matmul(
                        cell["pv"][:, 0:GC],
                        xfull[:, k, sb * P:(sb + 1) * P], wv_sb[:, k, :],
                        start=(k == 0), stop=(k == KD - 1))

                def copy(cell=cell, sb=sb):
                    nc.vector.tensor_copy(
                        out=v_sb[:, sb, :, 0:HD],
                        in_=cell["pv"][:, 0:GC].rearrange(
                            "p (h d) -> p h d", h=GH))

                for k in range(KD):
                    f0.add(lambda k=k: alloc_mm(k))
                f0.add(copy)

            # pair-0 prerequisites per chunk, in ascending-chunk order.
            # Q/K gate the chunk's scores; each V block gets its own marker
            # so the first strips start before the V projections finish
            # (those then fill the strip windows).
            for ch in range(NCH):
                _proj_chunk(0, 0, ch)
                f0.add_marker(("q0", ch))
                _proj_chunk(1, 0, ch)
                f0.add_marker(("k0", ch))
                for s4 in range(CHW // P):
                    _v_block(ch * (CHW // P) + s4)
                    f0.add_marker(("v", ch * (CHW // P) + s4))
            # pair-1 Q/K projections, drained on demand per pair-1 chunk
            for ch in range(NCH):
                _proj_chunk(0, 1, ch)
                _proj_chunk(1, 1, ch)
                f0.add_marker(("qk1", ch))

            def _pre0(c):
                # only the chunk's Q projection gates its first strips; its
                # K chunk is needed no earlier than the diagonal groups and
                # drains on demand below, riding the windows until then
                f0.drain_until(("q0", c))

            def _need_v(jb):
                f0.drain_until(("k0", jb // 4))
                f0.drain_until(("v", jb))

            _emit_pair_attention(tc, 0, pools, tensors, f0, None,
                                 pre_chunk=_pre0, pre_group=_need_v)

            # pair-1: qk1 projections first (they gate the next chunk's
            # boundary), then outproj fillers (deadline is only the tail)
            f1 = _Fillers()
            chain = _FillerChain(f0, f1)

            ysb3 = [{} for _ in range(CHW // P)]

            def _outproj_gc0_prefill():
                # the pair-0 half of the final chunk's output projection is
                # computable as soon as pair-0 finished; run it under the
                # final chunk's strip windows so only the pair-1 half (plus
                # adds) remains after the last normalize chain
                for s4 in range(CHW // P):
                    sb = (NCH - 1) * (CHW // P) + s4
                    cell = ysb3[s4]

                    def alloc(cell=cell):
                        cell["ysb"] = ypool3.tile(
                            [P, D], BF16, tag="ysb3", name="ysb3")

                    f1.add(alloc)
                    for nch in range(2):
                        def mm0(cell=cell, sb=sb, nch=nch):
                            cell["py"] = ps_fill.tile(
                                [P, CHW], F32, tag="fill", name="fillpy0")
                            nc.tensor.matmul(
                                cell["py"], oT[:, 0, sb * P:(sb + 1) * P],
                                wo_sb[:, 0, nch * CHW:(nch + 1) * CHW],
                                start=True, stop=True)

                        def cp0(cell=cell, nch=nch):
                            nc.vector.tensor_copy(
                                out=cell["ysb"][:, nch * CHW:(nch + 1) * CHW],
                                in_=cell["py"])

                        f1.add(lambda mm0=mm0: mm0())
                        f1.add(cp0)

            def _outproj_chunk(c):
                final = (c == NCH - 1)
                for s4 in range(CHW // P):
                    sb = c * (CHW // P) + s4
                    if final:
                        cell = ysb3[s4]
                    else:
                        cell = {}

                        def alloc(cell=cell):
                            cell["ysb"] = ypool.tile(
                                [P, D], BF16, tag="ysb", name="ysb")

                        f1.add(alloc)
                    for nch in range(2):
                        if final:
                            # pair-1 half only; accumulate onto the staged
                            # pair-0 half. ScalarE is free after the last
                            # exp, but tensor_tensor isn't its repertoire,
                            # so adds stay on DVE.
                            def mm1(cell=cell, sb=sb, nch=nch):
                                cell["py"] = ps_fill.tile(
                                    [P, CHW], F32, tag="fill", name="fillpy1")
                                nc.tensor.matmul(
                                    cell["py"], oT[:, 1, sb * P:(sb + 1) * P],
                                    wo_sb[:, 1, nch * CHW:(nch + 1) * CHW],
                                    start=True, stop=True)

                            def addcp(cell=cell, nch=nch):
                                dst = cell["ysb"][:,
                                                  nch * CHW:(nch + 1) * CHW]
                                nc.vector.tensor_add(dst, dst, cell["py"])

                            f1.add(lambda mm1=mm1: mm1())
                            f1.add(addcp)
                        else:
                            def mm(gc, cell=cell, sb=sb, nch=nch):
                                if gc == 0:
                                    cell["py"] = ps_fill.tile(
                                        [P, CHW], F32, tag="fill",
                                        name="fillpy")
                                nc.tensor.matmul(
                                    cell["py"],
                                    oT[:, gc, sb * P:(sb + 1) * P],
                                    wo_sb[:, gc, nch * CHW:(nch + 1) * CHW],
                                    start=(gc == 0), stop=(gc == 1))

                            def cp(cell=cell, nch=nch):
                                nc.vector.tensor_copy(
                                    out=cell["ysb"][:,
                                                    nch * CHW:(nch + 1) * CHW],
                                    in_=cell["py"])

                            f1.add(lambda mm=mm: mm(0))
                            f1.add(lambda mm=mm: mm(1))
                            f1.add(cp)

                    def out_dma(cell=cell, sb=sb, split=final):
                        if split:
                            # tail: spread the final 512KB across both
                            # queues/rings so the end-of-kernel drain halves
                            nc.sync.dma_start(
                                out=y[sb * P:(sb + 1) * P, 0:CHW],
                                in_=cell["ysb"][:, 0:CHW])
                            nc.scalar.dma_start(
                                out=y[sb * P:(sb + 1) * P, CHW:D],
                                in_=cell["ysb"][:, CHW:D])
                        else:
                            nc.sync.dma_start(
                                out=y[sb * P:(sb + 1) * P, :],
                                in_=cell["ysb"])

                    f1.add(out_dma)
                if c == NCH - 2:
                    _outproj_gc0_prefill()

            def _pre1(c):
                f0.drain_until(("qk1", c))

            _emit_pair_attention(tc, 1, pools, tensors, chain, _outproj_chunk,
                                 pre_chunk=_pre1)
            f1.drain()
            f0.drain()


def _fix_instruction_waits(nc):
    """Some lowered ISA structs (fp32r matmul LDW, DMA pseudo) carry at most
    one sync wait. Normalize: hoist excess waits onto NoOps inserted
    immediately before the instruction in the scheduled stream (same engine,
    so program order preserves the wait semantics)."""
    fixed = 0
    for blk in nc.m.functions[0].blocks:
        insts = blk.instructions
        idx = 0
        while idx < len(insts):
            inst = insts[idx]
            si = getattr(inst, "sync_info", None)
            if si is not None and len(si.on_wait) > 1:
                waits = list(si.on_wait)
                for j, wt in enumerate(waits[:-1]):
                    nop = mybir.InstNoOp(
                        name=f"I-wfix{fixed}-{j}-{inst.name}",
                        engine=inst.engine,
                        sync_info=mybir.SyncInfo(on_wait=[wt], on_update=[]))
                    insts.insert(idx, nop)
                    idx += 1
                inst.sync_info = mybir.SyncInfo(
                    on_wait=[waits[-1]], on_update=list(si.on_update))
                fixed += 1
            idx += 1
    return fixed


def _build():
    global _NC_CACHE
    if _NC_CACHE is None:
        nc = bass.Bass()
        with tile.TileContext(nc) as tc:
            _emit(tc)
        _fix_instruction_waits(nc)
        _NC_CACHE = nc
    return _NC_CACHE


def kernel(x, Wq, Wkv, Wo):
    global LAST_RESULTS
    x = np.asarray(x, dtype=np.float32)
    Wq = np.asarray(Wq, dtype=np.float32)
    Wkv = np.asarray(Wkv, dtype=np.float32)
    Wo = np.asarray(Wo, dtype=np.float32)

    nc = _build()
    bf = ml_dtypes.bfloat16
    in_maps = []
    for c in range(8):
        b, g = divmod(c, 4)
        cs = slice(GC * g, GC * (g + 1))
        in_maps.append({
            "xT": np.ascontiguousarray(x[b].T).astype(bf),
            "wq": np.ascontiguousarray(Wq[:, cs]).astype(bf),
            "wk": np.ascontiguousarray(Wkv[:, 0:D][:, cs]).astype(bf),
            "wv": np.ascontiguousarray(Wkv[:, D:2 * D][:, cs]).astype(bf),
            "wo": np.ascontiguousarray(Wo[cs, :]).astype(bf),
        })

    trace = os.environ.get("ATTN_KERNEL_TRACE", "0") == "1"
    res = run_bass_kernel_spmd(nc, in_maps, list(range(8)), trace=trace)
    LAST_RESULTS = res

    out = np.zeros((B, S, D), dtype=np.float32)
    for c in range(8):
        b = c // 4
        out[b] += res.results[c]["y"].astype(np.float32)
    return out


if __name__ == "__main__":
    rng = np.random.default_rng(0)
    s = 1.0 / np.sqrt(D)
    inputs = {
        "x": rng.standard_normal((B, S, D), dtype=np.float32),
        "Wq": rng.standard_normal((D, D), dtype=np.float32) * s,
        "Wkv": rng.standard_normal((D, 2 * D), dtype=np.float32) * s,
        "Wo": rng.standard_normal((D, D), dtype=np.float32) * s,
    }
    out = kernel(**inputs)
    print("out", out.shape, out.dtype, float(np.abs(out).mean()))


# revision 56
# speedup vs baseline: 1.1892x; 1.1892x over previous
"""Multi-head causal attention (B=2, S=2048, D=1024, H=16) on 8 TRN2 NeuronCores.

Sharding: core c handles batch b = c//4 and head-group g = c%4 (4 heads, 256 dims).
Each core computes Q/K/V projections for its head group from x[b], runs causal
attention per head, and applies its 256 rows of Wo, producing a partial [S, D]
output (bf16). The host sums the 4 head-group partials per batch in fp32.

Device algorithm (per core); matmul operands bf16, accumulation fp32 in PSUM:
  warm-up matmul burst at body start un-throttles the PE HAM clock gate
  (kept small: long garbage bursts push the chip into the P0 power state
  and slow EVERY engine ~19%)
  qT/kT = Wq_g^T @ x^T, stored [64*2, pair, S] (head dims on partitions)
  v     = x @ Wv_g, stored per 128-seq block with an appended ones column
  attention runs per head-pair, both pairs ascending chunk order, with the two
  heads interleaved per 512-wide i-chunk:
    S^T[j,i] strips via matmul(lhsT=kT_block, rhs=qT_chunk); the two heads'
    matmuls are issued back-to-back on disjoint PE row groups (K=64 row
    pairing) so they run concurrently; diagonal strips narrowed to the
    causally-valid column range
    P~^T = exp(scale * S^T) (ScalarE, 2 strips per instruction), diagonal
    blocks masked with an upper-triangular 0/1 multiply
    O'^T[65, i] += V'_j^T @ P~^T_j  (PSUM accumulate; row 64 = softmax denom)
    per chunk, both heads: denominators copied out with the numerators,
    reciprocal'd lane-parallel via a DRAM reshape bounce, broadcast with a
    stride-0 DRAM read, then O^T = num * recip (DVE); head 1 of the pair
    lands at partitions 0-63 and is shifted to 64-127 with a SBUF->SBUF DMA
  y = O @ Wo_g (lhsT = O^T tiles), PSUM copied out bf16, DMA out.

All TensorE work besides the attention strips is interleaved as "fillers"
between strip groups: pair-0 windows consume the pair-0 Q/K/V projections
(with forced drains before the chunk that needs them); pair-1 windows consume
the pair-1 Q/K projections (drained on demand per chunk) and the output
projection, emitted per chunk as soon as that chunk's oT is normalized. For
the final chunk the pair-0 half of the output projection is pre-staged under
its strip windows, so the post-attention tail is only that chunk's normalize
chain plus the pair-1 matmuls, adds, and split-queue y DMAs.

Queue discipline (the big scheduling hazards found by tracing):
  - every DMA issue costs ~0.7us of the issuing engine's sequencer and ring
    backpressure can block that sequencer outright, so the scalar queue
    (whose sequencer carries the exp stream) gets no bulk loads: x rides
    sync, weights ride GpSimd SWDGE; scalar only takes tail-phase DMAs
    after the exp stream has drained
  - ScalarE/DVE are strict FIFOs: a copy that waits on late PE work blocks
    everything behind it, so mid-stream PSUM->SBUF copies stay on DVE and
    ScalarE only takes copies in exp-idle slots (the tail)
"""

import os
from collections import deque

import ml_dtypes
import numpy as np

import concourse.bass as bass
import concourse.mybir as mybir
import concourse.tile as tile
from concourse.bass_utils import run_bass_kernel_spmd
from concourse.masks import make_upper_triangular

F32 = mybir.dt.float32
BF16 = mybir.dt.bfloat16

B, S, D, H = 2, 2048, 1024, 16
HD = 64                     # head dim
GH = 4                      # heads per core
GC = GH * HD                # 256 projection cols per core
P = 128
KD = D // P                 # 8 contraction chunks for projections
NSB = S // P                # 16 seq blocks
CHW = 512                   # i-chunk width
NCH = S // CHW              # 4 i-chunks
SCALE = HD ** -0.5
NWARM = 40                  # HAM warm-up matmuls (~4.3us cold = one SHORT window)

_NC_CACHE = None
LAST_RESULTS = None         # BassKernelResults of the most recent run (for test.py)


class _Fillers:
    """Queue of small emission closures (1-2 TensorE ops each) drained
    between attention strip groups to keep the PE busy while ScalarE
    works through the exp stream. Markers let the consumer force-drain
    the prefix a dependent phase needs."""

    def __init__(self):
        self.q = deque()
        self.seen = set()

    def add(self, fn):
        self.q.append(fn)

    def add_marker(self, key):
        self.q.append(key)

    def _emit_one(self):
        item = self.q.popleft()
        if callable(item):
            item()
            return None
        self.seen.add(item)
        return item

    def step(self, n):
        done = 0
        while done < n and self.q:
            if self._emit_one() is None:
                done += 1

    def drain_until(self, key):
        if key in self.seen:
            return
        while self.q:
            if self._emit_one() == key:
                return

    def drain(self):
        while self.q:
            self._emit_one()


class _FillerChain:
    """step() prefers the primary queue, falls back to the secondary."""

    def __init__(self, primary, secondary):
        self.primary = primary
        self.secondary = secondary

    def step(self, n):
        done = 0
        while done < n:
            if self.primary.q:
                if self.primary._emit_one() is None:
                    done += 1
            elif self.secondary.q:
                if self.secondary._emit_one() is None:
                    done += 1
            else:
                return


def _emit_pair_attention(tc, pair, pools, tensors, fillers, emit_outproj,
                         pre_chunk=None, pre_group=None):
    nc = tc.nc
    ps_sc, ps_pv, ps_fill, dpool, ppool, npool, opool = pools
    qT, kT, v_sb, oT, trimask, ones_row = tensors

    for c in range(NCH):
        njb = 4 * c + 4
        if pre_chunk is not None:
            pre_chunk(c)
        pvacc0 = ps_pv.tile([HD + 1, CHW], F32, tag="pv0", name="pvacc0")
        pvacc1 = ps_pv.tile([HD + 1, CHW], F32, tag="pv1", name="pvacc1")
        pvacc = {0: pvacc0, 1: pvacc1}
        # strip tasks, heads interleaved so paired score matmuls are adjacent
        tasks = [(hp, jb) for jb in range(njb) for hp in (0, 1)]
        for g0 in range(0, len(tasks), 2):
            group = tasks[g0:g0 + 2]
            if pre_group is not None:
                for _, jb in group:
                    pre_group(jb)
            sc = ps_sc.tile([P, 2, CHW], F32, tag="sc")
            pt = ppool.tile([P, 2, CHW], BF16, tag="pt")
            for t, (hp, jb) in enumerate(group):
                bp = hp * HD
                tl = max(0, jb - 4 * c) * P
                nc.tensor.matmul(
                    sc[:, t, tl:],
                    kT[bp:bp + HD, pair, jb * P:(jb + 1) * P],
                    qT[bp:bp + HD, pair, c * CHW + tl:(c + 1) * CHW])
            tlg = max(0, group[0][1] - 4 * c) * P
            nc.scalar.activation(
                pt[:, :len(group), tlg:], sc[:, :len(group), tlg:],
                mybir.ActivationFunctionType.Exp, scale=SCALE)
            for t, (hp, jb) in enumerate(group):
                if jb >= 4 * c:               # diagonal block: causal mask
                    tl = (jb - 4 * c) * P
                    nc.vector.tensor_mul(
                        pt[:, t, tl:tl + P], pt[:, t, tl:tl + P], trimask)
            for t, (hp, jb) in enumerate(group):
                h = pair * 2 + hp
                tl = max(0, jb - 4 * c) * P
                nc.tensor.matmul(
                    pvacc[hp][:, tl:], v_sb[:, jb, h, :], pt[:, t, tl:],
                    start=(jb == 0), stop=(jb == njb - 1))
            fillers.step(7)

        if pair == 1 and c == NCH - 1:
            # the normalize chain below is the only thing between the last
            # strip and the final outproj burst; keep the PE's HAM clock
            # gate open across that stall with a few dependency-free matmuls
            wt = ps_fill.tile([P, CHW], F32, tag="fill", name="tailwarm")
            for w in range(24):
                nc.tensor.matmul(
                    wt[:, (w % 4) * P:(w % 4 + 1) * P], trimask, trimask,
                    start=True, stop=True)

        # per-chunk normalize for both heads: copy num/denom out of PSUM,
        # lane-parallel reciprocal via DRAM reshape, broadcast, multiply
        dden = dpool.tile([2, CHW], F32, tag="dden")
        onums = {}
        for hp in (0, 1):
            onum = opool.tile([HD + 1, CHW], F32, tag=f"on{hp}")
            nc.vector.tensor_copy(out=onum, in_=pvacc[hp])
            nc.sync.dma_start(
                out=dden[hp:hp + 1, :], in_=onum[HD:HD + 1, :])
            onums[hp] = onum
        nel = 2 * CHW // P                    # 8 elems/lane
        rv = npool.tile([P, nel], F32, tag="recp")
        nc.sync.dma_start(out=rv, in_=bass.AP(
            tensor=dden.tensor, offset=dden.offset, ap=[[nel, P], [1, nel]]))
        nc.vector.reciprocal(out=rv, in_=rv)
        drec = dpool.tile([2, CHW], F32, tag="drec")
        nc.sync.dma_start(out=bass.AP(
            tensor=drec.tensor, offset=drec.offset,
            ap=[[nel, P], [1, nel]]), in_=rv)
        cs = slice(c * CHW, (c + 1) * CHW)
        # after the last exp the scalar queue is free: let the final chunk's
        # broadcast/shift DMAs ride it in parallel with the sync queue
        at_tail = (pair == 1 and c == NCH - 1)
        for hp in (0, 1):
            bcr = npool.tile([HD, CHW], F32, tag="bcr")
            eng = nc.scalar if (at_tail and hp == 1) else nc.sync
            eng.dma_start(out=bcr, in_=bass.AP(
                tensor=drec.tensor, offset=drec.offset + hp * CHW,
                ap=[[0, HD], [1, CHW]]))
            if hp == 0:
                nc.vector.tensor_mul(
                    oT[0:HD, pair, cs], onums[hp][0:HD, :], bcr)
            else:
                tmp = npool.tile([HD, CHW], BF16, tag="otmp")
                nc.vector.tensor_mul(tmp, onums[hp][0:HD, :], bcr)
                eng2 = nc.scalar if at_tail else nc.sync
                eng2.dma_start(out=oT[HD:P, pair, cs], in_=tmp)
        if emit_outproj is not None:
            emit_outproj(c)


def _emit(tc):
    nc = tc.nc
    xT = nc.dram_tensor("xT", [D, S], BF16, kind="ExternalInput")
    wq = nc.dram_tensor("wq", [D, GC], BF16, kind="ExternalInput")
    wk = nc.dram_tensor("wk", [D, GC], BF16, kind="ExternalInput")
    wv = nc.dram_tensor("wv", [D, GC], BF16, kind="ExternalInput")
    wo = nc.dram_tensor("wo", [GC, D], BF16, kind="ExternalInput")
    y = nc.dram_tensor("y", [S, D], BF16, kind="ExternalOutput")

    xT_t = xT[:].rearrange("(o p) s -> p o s", p=P)      # [128, 8, S]
    wq_t = wq[:].rearrange("(o p) c -> p o c", p=P)      # [128, 8, 256]
    wk_t = wk[:].rearrange("(o p) c -> p o c", p=P)
    wv_t = wv[:].rearrange("(o p) c -> p o c", p=P)
    wo_t = wo[:].rearrange("(o p) n -> p o n", p=P)      # [128, 2, 1024]

    from contextlib import ExitStack

    with ExitStack() as top:
        persist = top.enter_context(tc.tile_pool(name="persist", bufs=1))

        trimask = persist.tile([P, P], BF16)             # 1.0 where j<=i else 0
        make_upper_triangular(nc, trimask, val=1.0, diag=True)
        ones_bf = persist.tile([P, 1], BF16)
        nc.vector.memset(ones_bf, 1.0)
        ones_row = persist.tile([1, HD], BF16)           # outer-product lhsT
        nc.vector.memset(ones_row, 1.0)

        wq_sb = persist.tile([P, KD, GC], BF16)
        wk_sb = persist.tile([P, KD, GC], BF16)
        wv_sb = persist.tile([P, KD, GC], BF16)
        wo_sb = persist.tile([P, 2, D], BF16)
        xfull = persist.tile([P, KD, S], BF16)
        # x rides the sync queue, weights ride GpSimd SWDGE, and the scalar
        # queue carries NO bulk loads: DMA-ring backpressure blocks the
        # issuing sequencer, and the scalar sequencer must reach the exp
        # stream quickly
        for g in range(KD // 2):
            nc.sync.dma_start(
                out=xfull[:, 2 * g:2 * g + 2, 0:CHW],
                in_=xT_t[:, 2 * g:2 * g + 2, 0:CHW])
        for half in range(2):
            ks = slice(4 * half, 4 * half + 4)
            nc.gpsimd.dma_start(out=wq_sb[:, ks, :], in_=wq_t[:, ks, :])
            nc.gpsimd.dma_start(out=wk_sb[:, ks, :], in_=wk_t[:, ks, :])
        for half in range(2):
            ks = slice(4 * half, 4 * half + 4)
            nc.gpsimd.dma_start(out=wv_sb[:, ks, :], in_=wv_t[:, ks, :])
        for ch in range(1, NCH):
            for g in range(KD // 2):
                nc.sync.dma_start(
                    out=xfull[:, 2 * g:2 * g + 2, ch * CHW:(ch + 1) * CHW],
                    in_=xT_t[:, 2 * g:2 * g + 2, ch * CHW:(ch + 1) * CHW])
        nc.gpsimd.dma_start(out=wo_sb[:, 0:1, :], in_=wo_t[:, 0:1, :])
        nc.gpsimd.dma_start(out=wo_sb[:, 1:2, :], in_=wo_t[:, 1:2, :])

        qT = persist.tile([P, 2, S], BF16)               # [pair-cols, pair, seq]
        kT = persist.tile([P, 2, S], BF16)
        v_sb = persist.tile([P, NSB, GH, HD + 1], BF16)  # ones col appended
        oT = persist.tile([P, 2, S], BF16)
        nc.vector.tensor_copy(
            out=v_sb[:, :, :, HD:HD + 1],
            in_=ones_bf[:, 0:1].to_broadcast((P, NSB, GH, 1)))

        tensors = (qT, kT, v_sb, oT, trimask, ones_row)

        # ---- attention with all projections as ordered fillers ----
        with ExitStack() as ph_b:
            ps_sc = ph_b.enter_context(
                tc.tile_pool(name="ps_sc", bufs=2, space="PSUM"))
            ps_pv = ph_b.enter_context(
                tc.tile_pool(name="ps_pv", bufs=1, space="PSUM"))
            ps_fill = ph_b.enter_context(
                tc.tile_pool(name="ps_fill", bufs=2, space="PSUM"))
            dpool = ph_b.enter_context(
                tc.tile_pool(name="dscr", bufs=4, space="DRAM"))
            ppool = ph_b.enter_context(tc.tile_pool(name="pstrip", bufs=5))
            npool = ph_b.enter_context(tc.tile_pool(name="norm", bufs=6))
            opool = ph_b.enter_context(tc.tile_pool(name="onum", bufs=3))
            ypool = ph_b.enter_context(tc.tile_pool(name="ystage", bufs=2))
            ypool3 = ph_b.enter_context(tc.tile_pool(name="ystage3", bufs=4))
            pools = (ps_sc, ps_pv, ps_fill, dpool, ppool, npool, opool)

            # HAM warm-up: ~4us of back-to-back cold matmuls so the PE clock
            # gate opens before the real projection stream begins
            wtile = ps_fill.tile([P, CHW], F32, tag="fill", name="warm")
            for w in range(NWARM):
                nc.tensor.matmul(
                    wtile[:, (w % 4) * P:(w % 4 + 1) * P], trimask, trimask,
                    start=True, stop=True)
            # preload the Exp activation table during the DMA lead-in so the
            # first real exp doesn't pay the ~1.3us table load
            etab = npool.tile([P, P], BF16, tag="etab")
            nc.scalar.activation(
                etab, trimask, mybir.ActivationFunctionType.Exp, scale=1.0)

            f0 = _Fillers()

            def _proj_chunk(which, pair_, ch):
                # which: 0=Q, 1=K; emits 8 accumulating matmuls + copy-out.
                # pair-1 copies land at chunk-boundary drains where the exp
                # FIFO has natural slack, so they ride ScalarE to relieve
                # DVE, which co-paces pair-1.
                cell = {}
                w_sb = wq_sb if which == 0 else wk_sb
                dst = qT if which == 0 else kT

                def alloc_mm(k, cell=cell, ch=ch, w_sb=w_sb, pair_=pair_):
                    if k == 0:
                        cell["p"] = ps_fill.tile(
                            [P, CHW], F32, tag="fill", name="fillqk")
                    nc.tensor.matmul(
                        cell["p"], w_sb[:, k, pair_ * P:(pair_ + 1) * P],
                        xfull[:, k, ch * CHW:(ch + 1) * CHW],
                        start=(k == 0), stop=(k == KD - 1))

                def copy(cell=cell, ch=ch, dst=dst, pair_=pair_):
                    nc.vector.tensor_copy(
                        out=dst[:, pair_, ch * CHW:(ch + 1) * CHW],
                        in_=cell["p"])

                for k in range(KD):
                    f0.add(lambda k=k: alloc_mm(k))
                f0.add(copy)

            def _v_block(sb):
                cell = {}

                def alloc_mm(k, cell=cell, sb=sb):
                    if k == 0:
                        cell["pv"] = ps_fill.tile(
                            [P, CHW], F32, tag="fill", name="fillpv")
                    nc.tensor.matmul(
                        cell["pv"][:, 0:GC],
                        xfull[:, k, sb * P:(sb + 1) * P], wv_sb[:, k, :],
                        start=(k == 0), stop=(k == KD - 1))

                def copy(cell=cell, sb=sb):
                    nc.vector.tensor_copy(
                        out=v_sb[:, sb, :, 0:HD],
                        in_=cell["pv"][:, 0:GC].rearrange(
                            "p (h d) -> p h d", h=GH))

                for k in range(KD):
                    f0.add(lambda k=k: alloc_mm(k))
                f0.add(copy)

            # pair-0 prerequisites per chunk, in ascending-chunk order.
            # Q/K gate the chunk's scores; each V block gets its own marker
            # so the first strips start before the V projections finish
            # (those then fill the strip windows).
            for ch in range(NCH):
                _proj_chunk(0, 0, ch)
                _proj_chunk(1, 0, ch)
                f0.add_marker(("pre0", ch))
                for s4 in range(CHW // P):
                    _v_block(ch * (CHW // P) + s4)
                    f0.add_marker(("v", ch * (CHW // P) + s4))
            # pair-1 Q/K projections, drained on demand per pair-1 chunk
            for ch in range(NCH):
                _proj_chunk(0, 1, ch)
                _proj_chunk(1, 1, ch)
                f0.add_marker(("qk1", ch))

            def _pre0(c):
                f0.drain_until(("pre0", c))

            def _need_v(jb):
                f0.drain_until(("v", jb))

            _emit_pair_attention(tc, 0, pools, tensors, f0, None,
                                 pre_chunk=_pre0, pre_group=_need_v)

            # pair-1: qk1 projections first (they gate the next chunk's
            # boundary), then outproj fillers (deadline is only the tail)
            f1 = _Fillers()
            chain = _FillerChain(f0, f1)

            ysb3 = [{} for _ in range(CHW // P)]

            def _outproj_gc0_prefill():
                # the pair-0 half of the final chunk's output projection is
                # computable as soon as pair-0 finished; run it under the
                # final chunk's strip windows so only the pair-1 half (plus
                # adds) remains after the last normalize chain
                for s4 in range(CHW // P):
                    sb = (NCH - 1) * (CHW // P) + s4
                    cell = ysb3[s4]

                    def alloc(cell=cell):
                        cell["ysb"] = ypool3.tile(
                            [P, D], BF16, tag="ysb3", name="ysb3")

                    f1.add(alloc)
                    for nch in range(2):
                        def mm0(cell=cell, sb=sb, nch=nch):
                            cell["py"] = ps_fill.tile(
                                [P, CHW], F32, tag="fill", name="fillpy0")
                            nc.tensor.matmul(
                                cell["py"], oT[:, 0, sb * P:(sb + 1) * P],
                                wo_sb[:, 0, nch * CHW:(nch + 1) * CHW],
                                start=True, stop=True)

                        def cp0(cell=cell, nch=nch):
                            nc.vector.tensor_copy(
                                out=cell["ysb"][:, nch * CHW:(nch + 1) * CHW],
                                in_=cell["py"])

                        f1.add(lambda mm0=mm0: mm0())
                        f1.add(cp0)

            def _outproj_chunk(c):
                final = (c == NCH - 1)
                for s4 in range(CHW // P):
                    sb = c * (CHW // P) + s4
                    if final:
                        cell = ysb3[s4]
                    else:
                        cell = {}

                        def alloc(cell=cell):
                            cell["ysb"] = ypool.tile(
                                [P, D], BF16, tag="ysb", name="ysb")

                        f1.add(alloc)
                    for nch in range(2):
                        if final:
                            # pair-1 half only; accumulate onto the staged
                            # pair-0 half. ScalarE is free after the last
                            # exp, but tensor_tensor isn't its repertoire,
                            # so adds stay on DVE.
                            def mm1(cell=cell, sb=sb, nch=nch):
                                cell["py"] = ps_fill.tile(
                                    [P, CHW], F32, tag="fill", name="fillpy1")
                                nc.tensor.matmul(
                                    cell["py"], oT[:, 1, sb * P:(sb + 1) * P],
                                    wo_sb[:, 1, nch * CHW:(nch + 1) * CHW],
                                    start=True, stop=True)

                            def addcp(cell=cell, nch=nch):
                                dst = cell["ysb"][:,
                                                  nch * CHW:(nch + 1) * CHW]
                                nc.vector.tensor_add(dst, dst, cell["py"])

                            f1.add(lambda mm1=mm1: mm1())
                            f1.add(addcp)
                        else:
                            def mm(gc, cell=cell, sb=sb, nch=nch):
                                if gc == 0:
                                    cell["py"] = ps_fill.tile(
                                        [P, CHW], F32, tag="fill",
                                        name="fillpy")
                                nc.tensor.matmul(
                                    cell["py"],
                                    oT[:, gc, sb * P:(sb + 1) * P],
                                    wo_sb[:, gc, nch * CHW:(nch + 1) * CHW],
                                    start=(gc == 0), stop=(gc == 1))

                            def cp(cell=cell, nch=nch):
                                nc.vector.tensor_copy(
                                    out=cell["ysb"][:,
                                                    nch * CHW:(nch + 1) * CHW],
                                    in_=cell["py"])

                            f1.add(lambda mm=mm: mm(0))
                            f1.add(lambda mm=mm: mm(1))
                            f1.add(cp)

                    def out_dma(cell=cell, sb=sb, split=final):
                        if split:
                            # tail: spread the final 512KB across both
                            # queues/rings so the end-of-kernel drain halves
                            nc.sync.dma_start(
                                out=y[sb * P:(sb + 1) * P, 0:CHW],
                                in_=cell["ysb"][:, 0:CHW])
                            nc.scalar.dma_start(
                                out=y[sb * P:(sb + 1) * P, CHW:D],
                                in_=cell["ysb"][:, CHW:D])
                        else:
                            nc.sync.dma_start(
                                out=y[sb * P:(sb + 1) * P, :],
                                in_=cell["ysb"])

                    f1.add(out_dma)
                if c == NCH - 2:
                    _outproj_gc0_prefill()

            def _pre1(c):
                f0.drain_until(("qk1", c))

            _emit_pair_attention(tc, 1, pools, tensors, chain, _outproj_chunk,
                                 pre_chunk=_pre1)
            f1.drain()
            f0.drain()


def _fix_instruction_waits(nc):
    """Some lowered ISA structs (fp32r matmul LDW, DMA pseudo) carry at most
    one sync wait. Normalize: hoist excess waits onto NoOps inserted
    immediately before the instruction in the scheduled stream (same engine,
    so program order preserves the wait semantics)."""
    fixed = 0
    for blk in nc.m.functions[0].blocks:
        insts = blk.instructions
        idx = 0
        while idx < len(insts):
            inst = insts[idx]
            si = getattr(inst, "sync_info", None)
            if si is not None and len(si.on_wait) > 1:
                waits = list(si.on_wait)
                for j, wt in enumerate(waits[:-1]):
                    nop = mybir.InstNoOp(
                        name=f"I-wfix{fixed}-{j}-{inst.name}",
                        engine=inst.engine,
                        sync_info=mybir.SyncInfo(on_wait=[wt], on_update=[]))
                    insts.insert(idx, nop)
                    idx += 1
                inst.sync_info = mybir.SyncInfo(
                    on_wait=[waits[-1]], on_update=list(si.on_update))
                fixed += 1
            idx += 1
    return fixed


def _build():
    global _NC_CACHE
    if _NC_CACHE is None:
        nc = bass.Bass()
        with tile.TileContext(nc) as tc:
            _emit(tc)
        _fix_instruction_waits(nc)
        _NC_CACHE = nc
    return _NC_CACHE


def kernel(x, Wq, Wkv, Wo):
    global LAST_RESULTS
    x = np.asarray(x, dtype=np.float32)
    Wq = np.asarray(Wq, dtype=np.float32)
    Wkv = np.asarray(Wkv, dtype=np.float32)
    Wo = np.asarray(Wo, dtype=np.float32)

    nc = _build()
    bf = ml_dtypes.bfloat16
    in_maps = []
    for c in range(8):
        b, g = divmod(c, 4)
        cs = slice(GC * g, GC * (g + 1))
        in_maps.append({
            "xT": np.ascontiguousarray(x[b].T).astype(bf),
            "wq": np.ascontiguousarray(Wq[:, cs]).astype(bf),
            "wk": np.ascontiguousarray(Wkv[:, 0:D][:, cs]).astype(bf),
            "wv": np.ascontiguousarray(Wkv[:, D:2 * D][:, cs]).astype(bf),
            "wo": np.ascontiguousarray(Wo[cs, :]).astype(bf),
        })

    trace = os.environ.get("ATTN_KERNEL_TRACE", "0") == "1"
    res = run_bass_kernel_spmd(nc, in_maps, list(range(8)), trace=trace)
    LAST_RESULTS = res

    out = np.zeros((B, S, D), dtype=np.float32)
    for c in range(8):
        b = c // 4
        out[b] += res.results[c]["y"].astype(np.float32)
    return out


if __name__ == "__main__":
    rng = np.random.default_rng(0)
    s = 1.0 / np.sqrt(D)
    inputs = {
        "x": rng.standard_normal((B, S, D), dtype=np.float32),
        "Wq": rng.standard_normal((D, D), dtype=np.float32) * s,
        "Wkv": rng.standard_normal((D, 2 * D), dtype=np.float32) * s,
        "Wo": rng.standard_normal((D, D), dtype=np.float32) * s,
    }
    out = kernel(**inputs)
    print("out", out.shape, out.dtype, float(np.abs(out).mean()))
